# revision 17
# baseline (speedup 1.0000x reference)
"""Causal self-attention (dense transformer block) on 8 Trainium2 NeuronCores.

Sharding (Megatron-style tensor parallel over heads):
  - 16 heads, 8 cores -> 2 heads/core. Each core computes the qkv projection
    for its 2 heads (column-sharded W_qkv), causal attention for those heads
    over all 4 batches, and a row-sharded c_proj partial (its 128 y-channels
    x its W_proj row-slice). The host sums the 8 partial outputs (the
    row-parallel unshard) and transposes back.

Key performance structure (v2):
  - All matmuls in bf16 (x, weights, q/k/v, exp-scores): PE runs at
    1 cycle/column; inputs are bf16 in DRAM so the x load is half the bytes.
  - Phase 2 is software-pipelined at issue level: QK(kt+3) is issued to the
    PE queue before PV(kt), so the PE never waits on the ACT Exp eviction,
    which otherwise resets the tensor-engine p-state every key tile.
  - c_proj for block i is issued after attention of block i+1 (one-block
    rotation), so the softmax-normalize chain (reciprocal+broadcast+mult)
    runs entirely under the next block's matmuls.
  - reciprocal_approx_fast (1 custom DVE op, ~5x faster than the iterative
    reciprocal) computes 1/rowsum; row sums come free from the PV matmul via
    a ones-column appended to V ([V|1]).
  - Causal masking: QK/Exp skip fully-masked 128-query subcolumns of
    diagonal key tiles (the skipped e_t region is zeroed by a strided
    memset); only the single triangular 128x128 subblock gets a mask
    multiply.
  - PSUM: 2 x [128,1024] pair slots (QK scores / qkv proj / c_proj) +
    4 x [*,512] bank slots (p_v/p_tr/p_y) = exactly 8 banks; 4 p_y slots
    keep two attention blocks in flight.
  - Output partials are written bf16 (halves the eviction + DMA cost);
    the host sums the 8 partials in fp32.
"""

import sys

sys.path.insert(0, "/opt/trn_rl_repo")

import numpy as np

N_CORES = 8
B, T, D = 4, 2048, 1024
H, DK = 16, 64
HPC = H // N_CORES            # heads per core = 2
CPC = HPC * DK                # channels per core = 128
ROWS = B * T                  # 8192
RT = 512                      # row-tile (free dim) for projections
N_RT = ROWS // RT             # 16
KTILE = 128                   # key tile
QB = 512                      # query block
N_QB = T // QB                # 4 query blocks per batch
N_KT_B = T // KTILE           # 16 key tiles per batch
NG = ROWS // KTILE            # 64 V groups
SCALE = 1.0 / np.sqrt(DK)


def build_program(use_bias=False):
    import concourse.bass as bass  # noqa: F401
    import concourse.mybir as mybir
    import concourse.tile as tile
    from concourse import bacc
    from concourse.masks import make_identity

    f32 = mybir.dt.float32
    f32r = mybir.dt.float32r
    bf16 = mybir.dt.bfloat16
    ACTF = mybir.ActivationFunctionType
    MUL = mybir.AluOpType.mult

    nc = bacc.Bacc(None, target_bir_lowering=False)
    with tile.TileContext(nc) as tc:
        with tc.tile_pool(name="dram", bufs=1, space="DRAM") as dram:
            xT = dram.tile([D, ROWS], bf16, kind="ExternalInput", name="xT", uniquify=False)
            wq = dram.tile([128, D], bf16, kind="ExternalInput", name="wq", uniquify=False)
            wk = dram.tile([128, D], bf16, kind="ExternalInput", name="wk", uniquify=False)
            wv = dram.tile([128, D], bf16, kind="ExternalInput", name="wv", uniquify=False)
            wp = dram.tile([CPC, D], bf16, kind="ExternalInput", name="wp", uniquify=False)
            bqkv = dram.tile([CPC, 3], f32, kind="ExternalInput", name="bqkv", uniquify=False)
            bp = dram.tile([128, D // 128], f32, kind="ExternalInput", name="bp", uniquify=False)
            outT = dram.tile([D, ROWS], bf16, kind="ExternalOutput", name="outT", uniquify=False)

            # ---------------- constants / weights in SBUF ----------------
            cst = tc.alloc_tile_pool(name="cst", bufs=1)
            wq_sb = cst.tile([128, D], bf16, name="wq_sb")
            wk_sb = cst.tile([128, D], bf16, name="wk_sb")
            wv_sb = cst.tile([128, D], bf16, name="wv_sb")
            wp_sb = cst.tile([CPC, D], bf16, name="wp_sb")
            for w_dram, w_sb in ((wq, wq_sb), (wk, wk_sb), (wv, wv_sb), (wp, wp_sb)):
                nc.sync.dma_start(out=w_sb[:], in_=w_dram[:])
            bqkv_sb = cst.tile([CPC, 3], f32, name="bqkv_sb")
            nc.sync.dma_start(out=bqkv_sb[:], in_=bqkv[:])
            bp_sb = cst.tile([128, D // 128], f32, name="bp_sb")
            nc.sync.dma_start(out=bp_sb[:], in_=bp[:])

            ident32 = cst.tile([128, 128], f32, name="ident32")
            make_identity(nc, ident32)
            ident = cst.tile([128, 128], bf16, name="ident")
            nc.vector.tensor_copy(ident[:], ident32[:])

            # triangular mask [128 k, 128 q]: keep where q >= k (bf16)
            mscratch = cst.tile([128, 128], f32, name="mscratch")
            nc.gpsimd.memset(mscratch[:], 1.0)
            nc.gpsimd.affine_select(
                out=mscratch[:],
                in_=mscratch[:],
                compare_op=mybir.AluOpType.is_ge,
                fill=0.0,
                base=0,
                pattern=[[1, 128]],
                channel_multiplier=-1,
            )
            msk = cst.tile([128, 128], bf16, name="msk")
            nc.vector.tensor_copy(msk[:], mscratch[:])

            # ---------------- long-lived activations ----------------
            # q^T at cols 0:ROWS, k^T at cols ROWS:2*ROWS  (ch-major, bf16)
            qkt_sb, _free_qk = tc.tile([CPC, 2 * ROWS], bf16, name="qkt_sb")
            # V tiles per key-tile g: [128 keys, 2x128]: per head
            # [1 | zeros63 | V64] -- ones first so row sums land on psum
            # partition 0 (reciprocal_approx_fast needs partition 0) and V
            # products land on partitions 64:128 (64-partition engine
            # accesses must start at partition 0 or 64)
            v_sb, _free_v = tc.tile([128, NG * 2 * 128], bf16, name="v_sb")
            v_hd = v_sb[:].rearrange("p (g h s) -> p g h s", h=HPC, s=128)
            nc.vector.memset(v_hd[:, :, :, 0:1], 1.0)
            nc.vector.memset(v_hd[:, :, :, 1:64], 0.0)

            # ---------------- pools ----------------
            xa = tc.alloc_tile_pool(name="xa", bufs=16)
            vts = tc.alloc_tile_pool(name="vts", bufs=3)
            att = tc.alloc_tile_pool(name="att", bufs=5)     # e_t [128,1024] bf16
            ynp = tc.alloc_tile_pool(name="ynp", bufs=2)
            bcp = tc.alloc_tile_pool(name="bcp", bufs=4)
            invp = tc.alloc_tile_pool(name="invp", bufs=4)
            osp = tc.alloc_tile_pool(name="osp", bufs=4)
            # PSUM (16KB/partition): tag "qk" 2 x [128,1024] (qkv proj pairs,
            # QK score pairs, c_proj pairs) + tag "py" 2 x 4KB (p_v/p_tr in
            # phase 1; the merged [65,1024] PV accumulator in phase 2 --
            # its own tag so c_proj rotation never collides with an
            # accumulating p_y)
            ps = tc.alloc_tile_pool(name="ps", bufs=2, space="PSUM")

            # ================= phase 1: qkv projections =================
            nkt = D // 128
            for rt in range(N_RT):
                rsl = slice(rt * RT, (rt + 1) * RT)
                xts = []
                for kt in range(nkt):
                    xt = xa.tile([128, RT], bf16, name="xt", tag="xt")
                    nc.sync.dma_start(out=xt[:], in_=xT[kt * 128:(kt + 1) * 128, rsl])
                    xts.append(xt)
                # q and k share one [128,1024] psum pair (separate bank halves)
                p_qk = ps.tile([CPC, 2 * RT], f32, name="p_qk", tag="qk")
                p_v = ps.tile([CPC, RT], f32, name="p_v", tag="py")
                for kt in range(nkt):
                    ksl = slice(kt * 128, (kt + 1) * 128)
                    st = kt == 0
                    sp = kt == nkt - 1
                    nc.tensor.matmul(p_qk[:, 0:RT], wq_sb[:, ksl], xts[kt][:], start=st, stop=sp)
                    nc.tensor.matmul(p_qk[:, RT:2 * RT], wk_sb[:, ksl], xts[kt][:], start=st, stop=sp)
                    nc.tensor.matmul(p_v[:], wv_sb[:, ksl], xts[kt][:], start=st, stop=sp)
                # evict Q^T and K^T in one strided activation
                qk_out = qkt_sb[:].rearrange("p (g r) -> p g r", g=2)[:, :, rsl]
                if use_bias:
                    nc.vector.tensor_scalar_add(qkt_sb[:, rsl], p_qk[:, 0:RT], bqkv_sb[:, 0:1])
                    nc.vector.tensor_scalar_add(
                        qkt_sb[:, ROWS + rt * RT:ROWS + (rt + 1) * RT],
                        p_qk[:, RT:2 * RT], bqkv_sb[:, 1:2],
                    )
                else:
                    nc.scalar.activation(
                        qk_out, p_qk[:].rearrange("p (g r) -> p g r", g=2), ACTF.Copy
                    )
                # V^T -> SBUF (with bias), then PE-transpose into V tiles
                vt_t = vts.tile([CPC, RT], bf16, name="vt_t", tag="vt")
                if use_bias:
                    nc.vector.tensor_scalar_add(vt_t[:], p_v[:], bqkv_sb[:, 2:3])
                else:
                    nc.scalar.activation(vt_t[:], p_v[:], ACTF.Copy)
                p_tr = ps.tile([128, RT], bf16, name="p_tr", tag="py")
                for c4 in range(RT // 128):
                    nc.tensor.transpose(
                        p_tr[:, c4 * 128:(c4 + 1) * 128],
                        vt_t[:, c4 * 128:(c4 + 1) * 128],
                        ident[:],
                    )
                # one strided copy into v_sb (ones columns pre-set)
                g0 = rt * (RT // 128)
                out_v = v_sb[:, g0 * 256:(g0 + 4) * 256].rearrange(
                    "p (g h s) -> p g h s", h=HPC, s=128
                )[:, :, :, 64:128]
                in_v = p_tr[:].rearrange("p (g h s) -> p g h s", h=HPC, s=64)
                nc.vector.tensor_copy(out_v, in_v)

            # ================= phase 2: causal attention + c_proj =======
            LOOK = 3

            def qk_issue(b, qb, kt):
                """Scores for key-tile kt of block (b,qb) -> one pair tile.
                Diagonal tiles only compute the unmasked q columns."""
                j = kt - (N_KT_B // N_QB) * qb
                c0 = max(0, j) * 128  # first live q column in the block
                kbase = ROWS + b * T + kt * KTILE
                q0 = b * T + qb * QB
                p_s = ps.tile([128, 2 * QB], f32, name="p_s", tag="qk")
                for h in range(HPC):
                    nc.tensor.matmul(
                        p_s[:, h * QB + c0:(h + 1) * QB],
                        qkt_sb[h * DK:(h + 1) * DK, kbase:kbase + KTILE],
                        qkt_sb[h * DK:(h + 1) * DK, q0 + c0:q0 + QB],
                        start=True, stop=True,
                    )
                return p_s, c0, j

            def exp_issue(p_s, c0, j):
                e_t = att.tile([128, 2 * QB], bf16, name="e_t", tag="et")
                if c0 > 0:
                    # zero the fully-masked q columns of both heads
                    ez = e_t[:].rearrange("p (h q) -> p h q", q=QB)[:, :, 0:c0]
                    nc.vector.memset(ez, 0.0)
                    ev = e_t[:].rearrange("p (h q) -> p h q", q=QB)[:, :, c0:QB]
                    pv_ = p_s[:].rearrange("p (h q) -> p h q", q=QB)[:, :, c0:QB]
                    nc.scalar.activation(ev, pv_, ACTF.Exp, scale=float(SCALE))
                else:
                    nc.scalar.activation(e_t[:], p_s[:], ACTF.Exp, scale=float(SCALE))
                if j >= 0:
                    # triangular 128x128 subblock mask (keep q >= k)
                    dsl = slice(j * 128, (j + 1) * 128)
                    ed = e_t[:].rearrange("p (h q) -> p h q", q=QB)[:, :, dsl]
                    nc.vector.tensor_tensor(
                        out=ed, in0=ed,
                        in1=msk[:][:, None, :].broadcast_to([128, HPC, 128]),
                        op=MUL,
                    )
                return e_t

            def pv_issue(b, kt, e_t, p_y, st, sp):
                g = b * N_KT_B + kt
                for h in range(HPC):
                    vbase = g * 256 + h * 128
                    nc.tensor.matmul(
                        p_y[:, h * QB:(h + 1) * QB], v_sb[:, vbase:vbase + 128],
                        e_t[:, h * QB:(h + 1) * QB],
                        start=st, stop=sp,
                    )

            def attn_block(b, qb):
                n_kt = (N_KT_B // N_QB) * (qb + 1)
                # both heads' PV accumulators in one pair-sized tile; row 0 of
                # each half is the softmax row sum (ones-first V layout)
                p_y = ps.tile([128, 2 * QB], f32, name="p_y", tag="py")
                pend = {}
                for kk in range(min(LOOK, n_kt)):
                    pend[kk] = qk_issue(b, qb, kk)
                for kt in range(n_kt):
                    p_s, c0, j = pend.pop(kt)
                    e_t = exp_issue(p_s, c0, j)
                    nk = kt + LOOK
                    if nk < n_kt:
                        pend[nk] = qk_issue(b, qb, nk)
                    pv_issue(b, kt, e_t, p_y, kt == 0, kt == n_kt - 1)
                return p_y

            def normalize_a(p_y):
                # reciprocal of both heads' row sums + partition broadcasts;
                # issued before c_proj so the DVE queue isn't blocked
                inv = invp.tile([1, 2 * QB], f32, name="inv", tag="inv")
                nc.vector.reciprocal_approx_fast(inv[:], p_y[0:1, :])
                bcs = []
                for h in range(HPC):
                    bc = bcp.tile([DK, QB], f32, name="bc", tag="bc")
                    nc.gpsimd.partition_broadcast(bc[:], inv[:, h * QB:(h + 1) * QB])
                    bcs.append(bc)
                return bcs

            def normalize_b(p_y, bcs):
                yn = ynp.tile([CPC, QB], bf16, name="yn", tag="yn")
                for h in range(HPC):
                    nc.vector.tensor_tensor(
                        out=yn[h * DK:(h + 1) * DK, :],
                        in0=p_y[64:128, h * QB:(h + 1) * QB], in1=bcs[h][:], op=MUL,
                    )
                return yn

            def cproj(b, qb, yn):
                qsl = slice(b * T + qb * QB, b * T + (qb + 1) * QB)
                for oc in range(D // 256):
                    p_o = ps.tile([128, 2 * QB], f32, name="p_o", tag="qk")
                    nc.tensor.matmul(
                        p_o[:, 0:QB], wp_sb[:, oc * 256:oc * 256 + 128], yn[:],
                        start=True, stop=True,
                    )
                    nc.tensor.matmul(
                        p_o[:, QB:2 * QB], wp_sb[:, oc * 256 + 128:oc * 256 + 256], yn[:],
                        start=True, stop=True,
                    )
                    ot = osp.tile([128, 2 * QB], bf16, name="ot", tag="ot")
                    if use_bias:
                        nc.vector.tensor_scalar_add(
                            ot[:, 0:QB], p_o[:, 0:QB], bp_sb[:, oc * 2:oc * 2 + 1]
                        )
                        nc.vector.tensor_scalar_add(
                            ot[:, QB:2 * QB], p_o[:, QB:2 * QB],
                            bp_sb[:, oc * 2 + 1:oc * 2 + 2],
                        )
                    else:
                        # split the eviction across engines so the pair slot
                        # frees fast and no single engine queue serializes
                        # gpsimd cannot read PSUM, so split between DVE and ACT
                        ev = [("v", "a"), ("v", "v"), ("a", "v"), ("v", "a")][oc]
                        for half, eng in enumerate(ev):
                            osl = slice(half * QB, (half + 1) * QB)
                            if eng == "v":
                                nc.vector.tensor_copy(ot[:, osl], p_o[:, osl])
                            else:
                                nc.scalar.activation(ot[:, osl], p_o[:, osl], ACTF.Copy)
                    nc.sync.dma_start(
                        out=outT[oc * 256:(oc + 1) * 256, qsl].rearrange(
                            "(g p) q -> p g q", p=128
                        ),
                        in_=ot[:].rearrange("p (g q) -> p g q", g=2),
                    )

            pending = None
            for b in range(B):
                for qb in range(N_QB):
                    p_y = attn_block(b, qb)
                    bcs = normalize_a(p_y)
                    if pending is not None:
                        cproj(*pending)
                    yn = normalize_b(p_y, bcs)
                    pending = (b, qb, yn)
            cproj(*pending)

            for _pool in (ps, osp, invp, bcp, ynp, att, vts, xa):
                _pool.release()
            _free_v(); _free_qk()
            cst.release()

    nc.compile()
    return nc


_CACHED = {}


def _get_program(use_bias=False):
    if use_bias not in _CACHED:
        _CACHED[use_bias] = build_program(use_bias)
    return _CACHED[use_bias]


def make_in_maps(x, W_qkv, b_qkv, W_proj, b_proj):
    import ml_dtypes

    bf16 = ml_dtypes.bfloat16
    x = np.asarray(x, dtype=np.float32)
    W_qkv = np.asarray(W_qkv, dtype=np.float32)
    b_qkv = np.asarray(b_qkv, dtype=np.float32)
    W_proj = np.asarray(W_proj, dtype=np.float32)
    b_proj = np.asarray(b_proj, dtype=np.float32)

    xT = np.ascontiguousarray(x.reshape(ROWS, D).T).astype(bf16)

    def pack_w(w):  # [D, CPC] -> SBUF layout [128, D]
        return np.ascontiguousarray(
            w.reshape(D // 128, 128, CPC).transpose(1, 0, 2).reshape(128, D)
        ).astype(bf16)

    in_maps = []
    for c in range(N_CORES):
        ch = c * CPC  # channel offset of this core's heads
        wq_c = pack_w(W_qkv[:, ch:ch + CPC])
        wk_c = pack_w(W_qkv[:, D + ch:D + ch + CPC])
        wv_c = pack_w(W_qkv[:, 2 * D + ch:2 * D + ch + CPC])
        wp_c = np.ascontiguousarray(W_proj[ch:ch + CPC, :]).astype(bf16)
        bqkv_c = np.stack(
            [b_qkv[ch:ch + CPC], b_qkv[D + ch:D + ch + CPC], b_qkv[2 * D + ch:2 * D + ch + CPC]],
            axis=1,
        ).astype(np.float32)
        # b_proj added once (core 0 only); partials are summed on host
        bp_c = (
            np.ascontiguousarray(b_proj.reshape(D // 128, 128).T)
            if c == 0
            else np.zeros((128, D // 128), np.float32)
        )
        in_maps.append(
            {
                "xT": xT,
                "wq": wq_c,
                "wk": wk_c,
                "wv": wv_c,
                "wp": wp_c,
                "bqkv": np.ascontiguousarray(bqkv_c),
                "bp": np.ascontiguousarray(bp_c.astype(np.float32)),
            }
        )
    return in_maps


def run(nc, in_maps, trace=False, trace_kwargs=None):
    from concourse.bass_utils import run_bass_kernel_spmd

    return run_bass_kernel_spmd(
        nc,
        in_maps,
        core_ids=list(range(N_CORES)),
        trace=trace,
        **(trace_kwargs or {}),
    )


def gather_output(results):
    acc = results[0]["outT"].astype(np.float32)
    for r in results[1:]:
        acc = acc + r["outT"].astype(np.float32)
    return np.ascontiguousarray(acc.T).reshape(B, T, D)


def kernel(x, W_qkv, b_qkv, W_proj, b_proj):
    use_bias = bool(np.any(np.asarray(b_qkv)) or np.any(np.asarray(b_proj)))
    nc = _get_program(use_bias)
    in_maps = make_in_maps(x, W_qkv, b_qkv, W_proj, b_proj)
    res = run(nc, in_maps, trace=False)
    return gather_output(res.results)
